# revision 1
# baseline (speedup 1.0000x reference)
"""BitNet ternary linear layer on 8 Trainium2 NeuronCores.

out[b, o] = (sum_i w[o,i] * round_clip(x[b,i]/act_scale)) * weight_scale * act_scale + bias[o]
  with w = unpack2bit(packed_weight) - 1   (codes c in {0..3} -> w in {-1..2})
  and  act_scale = max(absmax(x), 1e-5) / 127.

Strategy (tensor-parallel, column sharded over out_features):
 - Host: transpose packed_weight to [I/4, O] and slice O across 8 cores; put x
   in a PE-stationary-friendly layout. Pure layout prep; all 224 MiB of packed
   weight still stream through each core's HBM.
 - Device (per core, identical program):
   * quantize x on-chip: absmax -> r=127/absmax -> x_q = rne(x*r) (exact,
     magic-number rounding), kept in bf16 (integers <= 127, exact).
   * stream packed weight slices with a casting DMA (int32 -> uint8), which
     compacts the 1-byte payload 4:1 into SBUF.
   * unpack 2-bit planes with ONE fused DVE op per plane:
     (word >> 2k) & 0x03030303. The resulting bytes {0,1,2,3} are read as
     fp8e4 (e4m3) DENORMALS with exact values c * 2^-9 -> the PE multiplies
     them directly against the bf16 stationary x_q (mixed-dtype matmul).
   * the skinny (M=8) matmuls are packed 4-wide into the 128x128 PE array via
     column tiling (tile_position=(0, 32g)) so 4 output chunks compute
     concurrently in different column groups.
   * accumulate acc = sum_i c*x_q*2^-9 in PSUM (f32, exact), then epilogue
     out = acc*512*gamma - gamma*Sx + bias  with Sx[b] = sum_i x_q[b,i]
     (folds the code-minus-one into a rank-1 correction), gamma =
     weight_scale*act_scale.
"""

import os
import sys

sys.path.insert(0, "/opt/trn_rl_repo")

import numpy as np

import concourse.bacc as bacc
import concourse.mybir as mybir
from concourse import bass_isa
from concourse import tile
from concourse.bass_utils import run_bass_kernel_spmd

AluOp = mybir.AluOpType
dt = mybir.dt

O, I, B = 28672, 8192, 8
NCORES = 8
OS = O // NCORES          # 3584 out-features per core
J = I // 4                # 2048 packed words per out-feature
NJT = J // 128            # 16 j-tiles
MAGIC = 12582912.0        # 1.5 * 2^23: magic RNE round-to-integer constant

CH = 448                  # o-chunk size (col-tiled path): 8 chunks, 2 per group
NG = 4                    # PE column groups

_cache = {}
LAST_RESULTS = None       # test harness can inspect profiling info here


def _build(repeat=1, coltile=True, mode="full", compact="act"):
    # mode: "full" = real kernel; "dma" = weight stream only;
    #        "dmaplanes" = stream + DVE unpack only (perf bisection)
    # compact: "act" = raw HWDGE DMA + ScalarE int32->uint8 copy;
    #          "dma" = casting SWDGE DMA (slower stream, no ACT work)
    nc = bacc.Bacc("TRN2", target_bir_lowering=False, debug=False)

    pt = nc.dram_tensor("pt", [J, OS], dt.int32, kind="ExternalInput")
    xs = nc.dram_tensor("xs", [128, 512], dt.float32, kind="ExternalInput")
    biasr = nc.dram_tensor("biasr", [8, OS], dt.float32, kind="ExternalInput")
    ws = nc.dram_tensor("ws", [1, 1], dt.float32, kind="ExternalInput")
    out = nc.dram_tensor("out", [8, OS], dt.float32, kind="ExternalOutput")

    with tile.TileContext(nc) as tc:
        with (
            tc.tile_pool(name="io", bufs=2) as io,
            tc.tile_pool(name="wpool", bufs=3) as wpool,
            tc.tile_pool(name="plpool", bufs=2) as plpool,
            tc.tile_pool(name="opool", bufs=4) as opool,
            tc.tile_pool(name="ps", bufs=1, space="PSUM") as ps,
        ):
            xs_t = io.tile([128, 512], dt.float32)
            nc.sync.dma_start(xs_t[:], xs[:])
            if coltile:
                biasr_t = io.tile([128, OS], dt.float32)
                for g in range(NG):
                    nc.sync.dma_start(biasr_t[32 * g:32 * g + 8, :], biasr[:])
            else:
                biasr_t = io.tile([8, OS], dt.float32)
                nc.sync.dma_start(biasr_t[:], biasr[:])
            ws_t = io.tile([1, 1], dt.float32)
            nc.sync.dma_start(ws_t[:], ws[:])

            if mode == "planesonly":
                zt = io.tile([8, OS], dt.float32)
                nc.vector.memset(zt[:], 0.0)
                cb0 = io.tile([128, OS], dt.uint8)
                nc.gpsimd.dma_start(cb0[:], pt[0:128, :])
                cbi0 = cb0[:].bitcast(dt.int32)
                for _rep in range(repeat):
                    for jt in range(NJT):
                        for k in range(4):
                            pk = plpool.tile([128, OS // 4], dt.int32, tag=f"pk{k}")
                            nc.vector.tensor_scalar(
                                out=pk[:], in0=cbi0, scalar1=2 * k,
                                scalar2=0x03030303,
                                op0=AluOp.logical_shift_right,
                                op1=AluOp.bitwise_and,
                            )
                nc.sync.dma_start(out[:, :], zt[:])
                repeat = 0

            if mode not in ("full", "planesonly"):
                zt = io.tile([8, OS], dt.float32)
                nc.vector.memset(zt[:], 0.0)
                for _rep in range(repeat):
                    for jt in range(NJT):
                        if mode == "dmaraw":
                            cr = wpool.tile([128, OS], dt.int32, tag="cr")
                            nc.sync.dma_start(cr[:], pt[jt * 128:(jt + 1) * 128, :])
                            continue
                        cb = wpool.tile([128, OS], dt.uint8, tag="cb")
                        nc.gpsimd.dma_start(cb[:], pt[jt * 128:(jt + 1) * 128, :])
                        if mode == "dmaplanes":
                            cbi = cb[:].bitcast(dt.int32)
                            for k in range(4):
                                pk = plpool.tile([128, OS // 4], dt.int32,
                                                 tag=f"pk{k}")
                                nc.vector.tensor_scalar(
                                    out=pk[:], in0=cbi, scalar1=2 * k,
                                    scalar2=0x03030303,
                                    op0=AluOp.logical_shift_right,
                                    op1=AluOp.bitwise_and,
                                )
                nc.sync.dma_start(out[:, :], zt[:])
                repeat = 0   # skip the full-mode body below

            for _rep in range(repeat):
                # ---------- x quantization ----------
                am_p = io.tile([128, 1], dt.float32)
                nc.vector.tensor_reduce(
                    am_p[:], xs_t[:], axis=mybir.AxisListType.X, op=AluOp.max,
                    apply_absolute_value=True,
                )
                am = io.tile([128, 1], dt.float32)
                nc.gpsimd.partition_all_reduce(
                    am[:], am_p[:], channels=128, reduce_op=bass_isa.ReduceOp.absmax
                )
                nc.vector.tensor_scalar_max(am[:], am[:], 1e-5)

                # r = 127/absmax ; gamma = ws*absmax/127 ; g512 = gamma*512
                r = io.tile([128, 1], dt.float32)
                nc.vector.reciprocal(r[:], am[:])
                nc.vector.tensor_scalar_mul(r[:], r[:], 127.0)

                ws_b = io.tile([128, 1], dt.float32)
                nc.gpsimd.partition_broadcast(ws_b[:], ws_t[:])
                gamma = io.tile([128, 1], dt.float32)
                nc.vector.tensor_scalar(
                    out=gamma[:], in0=am[:], scalar1=1.0 / 127.0, scalar2=ws_b[:],
                    op0=AluOp.mult, op1=AluOp.mult,
                )
                g512 = io.tile([128, 1], dt.float32)
                nc.vector.tensor_scalar_mul(g512[:], gamma[:], 512.0)

                # x_q = rne(x*r) exactly, into bf16 (integers, exact)
                xq_f = io.tile([128, 512], dt.float32)
                nc.vector.tensor_scalar(
                    out=xq_f[:], in0=xs_t[:], scalar1=r[:], scalar2=MAGIC,
                    op0=AluOp.mult, op1=AluOp.add,
                )
                xq = io.tile([128, 512], dt.bfloat16)
                nc.vector.tensor_scalar(
                    out=xq[:], in0=xq_f[:], scalar1=MAGIC, scalar2=None,
                    op0=AluOp.subtract,
                )

                # Sx*gamma (rank-1 correction): partial sums over (jt,k) keep b,
                # then contract partitions against broadcast gamma on the PE.
                t_pb = io.tile([128, 8], dt.float32)
                nc.vector.tensor_reduce(
                    t_pb[:],
                    xq[:].rearrange("p (jt k b) -> p b (jt k)", jt=NJT, k=4, b=8),
                    axis=mybir.AxisListType.X, op=AluOp.add,
                )
                sxg_ps = ps.tile([128, 1], dt.float32)
                sxg = io.tile([128, 1], dt.float32)
                if coltile:
                    for g in range(NG):
                        nc.tensor.matmul(
                            sxg_ps[32 * g:32 * g + 8, :], t_pb[:], gamma[:],
                            start=True, stop=True, tile_position=(0, 32 * g),
                        )
                        nc.vector.tensor_copy(
                            sxg[32 * g:32 * g + 8, :], sxg_ps[32 * g:32 * g + 8, :]
                        )
                else:
                    nc.tensor.matmul(
                        sxg_ps[0:8, :], t_pb[:], gamma[:], start=True, stop=True
                    )
                    nc.vector.tensor_copy(sxg[0:8, :], sxg_ps[0:8, :])

                # ---------- main loop: stream weights, unpack, matmul ----------
                if coltile:
                    acc = ps.tile([128, 1024], dt.float32)   # 2 banks; chunk cc at cc*512
                else:
                    acc = ps.tile([8, OS], dt.float32)
                for jt in range(NJT):
                    use_raw = (compact == "act") or (
                        compact == "mix" and jt % 4 == 3
                    )
                    if use_raw:
                        cr = wpool.tile([128, OS], dt.int32, tag="cr")
                        nc.sync.dma_start(cr[:], pt[jt * 128:(jt + 1) * 128, :])
                        cb = wpool.tile([128, OS], dt.uint8, tag="cb")
                        nc.scalar.copy(cb[:], cr[:])
                    else:
                        cb = wpool.tile([128, OS], dt.uint8, tag="cb")
                        nc.gpsimd.dma_start(cb[:], pt[jt * 128:(jt + 1) * 128, :])
                    cbi = cb[:].bitcast(dt.int32)          # [128, OS/4]
                    for k in range(4):
                        pk = plpool.tile([128, OS // 4], dt.int32, tag=f"pk{k}")
                        if k == 0:
                            nc.vector.tensor_scalar(
                                out=pk[:], in0=cbi, scalar1=0x03030303, scalar2=None,
                                op0=AluOp.bitwise_and,
                            )
                        else:
                            nc.vector.tensor_scalar(
                                out=pk[:], in0=cbi, scalar1=2 * k, scalar2=0x03030303,
                                op0=AluOp.logical_shift_right, op1=AluOp.bitwise_and,
                            )
                        pk8 = pk[:].bitcast(dt.float8e4)   # bytes c -> denormal c*2^-9
                        lhsT = xq[:, (jt * 4 + k) * 8:(jt * 4 + k + 1) * 8]
                        first = (jt == 0 and k == 0)
                        last = (jt == NJT - 1 and k == 3)
                        if coltile:
                            for cc in range(2):
                                for g in range(NG):
                                    m = 2 * g + cc          # global o-chunk
                                    nc.tensor.matmul(
                                        acc[32 * g:32 * g + 8,
                                            cc * 512:cc * 512 + CH],
                                        lhsT,
                                        pk8[:, m * CH:(m + 1) * CH],
                                        start=first, stop=last,
                                        tile_position=(0, 32 * g),
                                    )
                        else:
                            for oc in range(OS // 512):
                                nc.tensor.matmul(
                                    acc[:, oc * 512:(oc + 1) * 512],
                                    lhsT,
                                    pk8[:, oc * 512:(oc + 1) * 512],
                                    start=first, stop=last,
                                )

                # ---------- epilogue ----------
                if coltile:
                    for cc in range(2):
                        ot = opool.tile([128, CH], dt.float32, tag="ot")
                        for g in range(NG):
                            m = 2 * g + cc
                            sl = slice(32 * g, 32 * g + 8)
                            nc.vector.tensor_scalar(
                                out=ot[sl, :],
                                in0=acc[sl, cc * 512:cc * 512 + CH],
                                scalar1=g512[sl, :], scalar2=sxg[sl, :],
                                op0=AluOp.mult, op1=AluOp.subtract,
                            )
                            nc.vector.tensor_tensor(
                                out=ot[sl, :], in0=ot[sl, :],
                                in1=biasr_t[sl, m * CH:(m + 1) * CH], op=AluOp.add,
                            )
                            nc.sync.dma_start(out[:, m * CH:(m + 1) * CH], ot[sl, :])
                else:
                    for oc in range(OS // 512):
                        sl = slice(oc * 512, (oc + 1) * 512)
                        ot = opool.tile([8, 512], dt.float32, tag="ot")
                        nc.vector.tensor_scalar(
                            out=ot[:], in0=acc[0:8, sl], scalar1=g512[0:8, :],
                            scalar2=sxg[0:8, :],
                            op0=AluOp.mult, op1=AluOp.subtract,
                        )
                        nc.vector.tensor_tensor(
                            out=ot[:], in0=ot[:], in1=biasr_t[:, sl], op=AluOp.add
                        )
                        nc.sync.dma_start(out[:, sl], ot[:])

    nc.compile()
    return nc


def kernel(x, packed_weight, weight_scale, bias):
    global LAST_RESULTS
    repeat = int(os.environ.get("BITNET_REPEAT", "1"))
    coltile = os.environ.get("BITNET_COLTILE", "1") != "0"
    compact = os.environ.get("BITNET_COMPACT", "mix")
    key = (repeat, coltile, compact)
    if key not in _cache:
        _cache[key] = _build(repeat, coltile, compact=compact)
    nc = _cache[key]

    x = np.asarray(x, dtype=np.float32)
    packed_weight = np.asarray(packed_weight, dtype=np.int32)
    weight_scale = np.asarray(weight_scale, dtype=np.float32)
    bias = np.asarray(bias, dtype=np.float32)

    # x -> stationary layout [p, (jt k b)]
    xs_np = np.ascontiguousarray(
        x.reshape(B, NJT, 128, 4).transpose(2, 1, 3, 0)
    ).reshape(128, 512)
    ws_np = weight_scale.reshape(1, 1)

    in_maps = []
    for c in range(NCORES):
        sl = slice(c * OS, (c + 1) * OS)
        ptc = np.ascontiguousarray(packed_weight[sl, :].T)       # [J, OS]
        biasc = np.ascontiguousarray(
            np.broadcast_to(bias[sl][None, :], (8, OS))
        )
        in_maps.append({"pt": ptc, "xs": xs_np, "biasr": biasc, "ws": ws_np})

    res = run_bass_kernel_spmd(nc, in_maps, list(range(NCORES)))
    LAST_RESULTS = res
    return np.concatenate(
        [np.asarray(res.results[c]["out"]) for c in range(NCORES)], axis=1
    ).reshape(B, O)



# revision 2
# speedup vs baseline: 2.7616x; 2.7616x over previous
"""BitNet ternary linear layer on 8 Trainium2 NeuronCores.

out[b, o] = (sum_i w[o,i] * round_clip(x[b,i]/act_scale)) * weight_scale * act_scale + bias[o]
  with w = unpack2bit(packed_weight) - 1   (codes c in {0..3} -> w in {-1..2})
  and  act_scale = max(absmax(x), 1e-5) / 127.

v2 strategy (tensor-parallel, column sharded over out_features):
 - The int32 packed_weight words only carry 8 payload bits (4x 2-bit codes,
   values <= 170). Host casts to uint8 losslessly and transposes to [I/4, OS]
   per core -> device HBM traffic drops 4x (29.4MB -> 7.34MB per core,
   ~20.4us at the ~360GB/s per-core DMA roofline).
 - Device (per core, identical program):
   * quantize x on-chip: absmax -> r=127/absmax -> x_q = rne(x*r) in bf16.
   * stream uint8 weight j-tiles; unpack 2-bit planes on DVE in BLK-jtile
     blocks: one fused tensor_scalar (word >> 2k) & 0x03030303 per plane,
     FD=BLK*896 int32 (write-port bound: 8B/cycle/lane).
   * planes feed the PE as fp8e4 DENORMALS (bytes {0..3} = c*2^-9 exact)
     against the bf16 stationary x_q, col-tiled 4-wide (tile_position).
   * acc in PSUM f32 (exact); epilogue merged over [128,1024]:
     out = acc*512*gamma - gamma*Sx + bias, gamma = weight_scale*act_scale.
"""

import os
import sys

sys.path.insert(0, "/opt/trn_rl_repo")

import numpy as np

import concourse.bacc as bacc
import concourse.mybir as mybir
from concourse import bass_isa
from concourse import tile
from concourse.bass_utils import run_bass_kernel_spmd

AluOp = mybir.AluOpType
dt = mybir.dt

O, I, B = 28672, 8192, 8
NCORES = 8
OS = O // NCORES          # 3584 out-features per core
JB = I // 4               # 2048 packed bytes per out-feature
NJT = JB // 128           # 16 j-tiles
MAGIC = 12582912.0        # 1.5 * 2^23: magic RNE round-to-integer constant

CH = 448                  # o-chunk size: 8 chunks, (g, cc) -> m = 2g+cc
NG = 4                    # PE column groups

_cache = {}
LAST_RESULTS = None       # test harness can inspect run results here


def _build(repeat=1, mode="full", blk=4):
    # mode: "full" = real kernel
    #       "dma"  = weight stream only          (DMA rate)
    #       "dmaplanes" = stream + DVE unpack    (max(DMA, DVE))
    #       "planes" = unpack from resident tile (DVE rate)
    #       "pe"   = matmuls from resident plane (PE rate)
    #       "mm"   = stream + matmuls, no unpack (max(DMA, PE))
    nc = bacc.Bacc("TRN2", target_bir_lowering=False, debug=False)
    NBLK = NJT // blk
    WT = blk * OS             # block tile bytes per partition

    wb = nc.dram_tensor("wb", [JB, OS], dt.uint8, kind="ExternalInput")
    xs = nc.dram_tensor("xs", [128, 512], dt.float32, kind="ExternalInput")
    biasc = nc.dram_tensor("biasc", [128, 1024], dt.float32, kind="ExternalInput")
    ws = nc.dram_tensor("ws", [1, 1], dt.float32, kind="ExternalInput")
    out = nc.dram_tensor("out", [8, OS], dt.float32, kind="ExternalOutput")

    with tile.TileContext(nc) as tc:
        with (
            tc.tile_pool(name="io", bufs=2) as io,
            tc.tile_pool(name="wpool", bufs=3) as wpool,
            tc.tile_pool(name="plpool", bufs=3) as plpool,
            tc.tile_pool(name="opool", bufs=2) as opool,
            tc.tile_pool(name="ps", bufs=2, space="PSUM") as ps,
        ):
            xs_t = io.tile([128, 512], dt.float32)
            nc.gpsimd.dma_start(xs_t[:], xs[:])
            biasc_t = io.tile([128, 1024], dt.float32)
            nc.gpsimd.dma_start(biasc_t[:], biasc[:])
            ws_t = io.tile([1, 1], dt.float32)
            nc.gpsimd.dma_start(ws_t[:], ws[:])

            if mode in ("planes", "pe"):
                wres = io.tile([128, WT], dt.uint8)
                for t in range(blk):
                    nc.sync.dma_start(wres[:, t * OS:(t + 1) * OS],
                                      wb[t * 128:(t + 1) * 128, :])
            if mode in ("pe", "mm"):
                gbuf = io.tile([128, OS], dt.uint8)
                nc.vector.memset(gbuf[:], 0.0)
                xqg = io.tile([128, 8], dt.bfloat16)
                nc.vector.memset(xqg[:], 1.0)
            if mode != "full":
                zt = io.tile([8, OS], dt.float32)
                nc.vector.memset(zt[:], 0.0)

            for _rep in range(repeat):
                if mode == "full":
                    # ---------- x quantization ----------
                    am_p = io.tile([128, 1], dt.float32)
                    nc.vector.tensor_reduce(
                        am_p[:], xs_t[:], axis=mybir.AxisListType.X, op=AluOp.max,
                        apply_absolute_value=True,
                    )
                    am = io.tile([128, 1], dt.float32)
                    nc.gpsimd.partition_all_reduce(
                        am[:], am_p[:], channels=128,
                        reduce_op=bass_isa.ReduceOp.absmax,
                    )
                    nc.vector.tensor_scalar_max(am[:], am[:], 1e-5)

                    r = io.tile([128, 1], dt.float32)
                    nc.vector.reciprocal(r[:], am[:])
                    nc.vector.tensor_scalar_mul(r[:], r[:], 127.0)

                    ws_b = io.tile([128, 1], dt.float32)
                    nc.gpsimd.partition_broadcast(ws_b[:], ws_t[:])
                    gamma = io.tile([128, 1], dt.float32)
                    nc.vector.tensor_scalar(
                        out=gamma[:], in0=am[:], scalar1=1.0 / 127.0,
                        scalar2=ws_b[:], op0=AluOp.mult, op1=AluOp.mult,
                    )
                    g512 = io.tile([128, 1], dt.float32)
                    nc.vector.tensor_scalar_mul(g512[:], gamma[:], 512.0)

                    xq_f = io.tile([128, 512], dt.float32)
                    nc.vector.tensor_scalar(
                        out=xq_f[:], in0=xs_t[:], scalar1=r[:], scalar2=MAGIC,
                        op0=AluOp.mult, op1=AluOp.add,
                    )
                    xq = io.tile([128, 512], dt.bfloat16)
                    nc.vector.tensor_scalar(
                        out=xq[:], in0=xq_f[:], scalar1=MAGIC, scalar2=None,
                        op0=AluOp.subtract,
                    )

                    # Sx*gamma rank-1 correction (codes = w+1)
                    t_pb = io.tile([128, 8], dt.float32)
                    nc.vector.tensor_reduce(
                        t_pb[:],
                        xq[:].rearrange("p (jt k b) -> p b (jt k)",
                                        jt=NJT, k=4, b=8),
                        axis=mybir.AxisListType.X, op=AluOp.add,
                    )
                    sxg_ps = ps.tile([128, 1], dt.float32, tag="sxg")
                    sxg = io.tile([128, 1], dt.float32)
                    for g in range(NG):
                        nc.tensor.matmul(
                            sxg_ps[32 * g:32 * g + 8, :], t_pb[:], gamma[:],
                            start=True, stop=True, tile_position=(0, 32 * g),
                        )
                        nc.vector.tensor_copy(
                            sxg[32 * g:32 * g + 8, :],
                            sxg_ps[32 * g:32 * g + 8, :],
                        )

                # ---------- main loop ----------
                if mode in ("full", "pe", "mm"):
                    acc = ps.tile([128, 1024], dt.float32, tag="acc")
                for b_ in range(NBLK):
                    if mode in ("full", "dma", "dmaplanes", "mm"):
                        wt = wpool.tile([128, WT], dt.uint8, tag="wt")
                        for t in range(blk):
                            jt = b_ * blk + t
                            nc.sync.dma_start(
                                wt[:, t * OS:(t + 1) * OS],
                                wb[jt * 128:(jt + 1) * 128, :],
                            )
                    if mode in ("dma",):
                        continue
                    src = wres if mode == "planes" else (
                        wt if mode in ("full", "dmaplanes") else None)
                    for k in range(4):
                        if mode in ("full", "dmaplanes", "planes"):
                            pk = plpool.tile([128, WT // 4], dt.int32, tag="pk")
                            if k == 0:
                                nc.vector.tensor_scalar(
                                    out=pk[:], in0=src[:].bitcast(dt.int32),
                                    scalar1=0x03030303, scalar2=None,
                                    op0=AluOp.bitwise_and,
                                )
                            else:
                                nc.vector.tensor_scalar(
                                    out=pk[:], in0=src[:].bitcast(dt.int32),
                                    scalar1=2 * k, scalar2=0x03030303,
                                    op0=AluOp.logical_shift_right,
                                    op1=AluOp.bitwise_and,
                                )
                            pk8 = pk[:].bitcast(dt.float8e4)  # [128, WT]
                        if mode in ("full", "pe", "mm"):
                            for t in range(blk):
                                jt = b_ * blk + t
                                if mode == "full":
                                    lhsT = xq[:, (jt * 4 + k) * 8:
                                              (jt * 4 + k + 1) * 8]
                                else:
                                    lhsT = xqg[:]
                                first = (b_ == 0 and k == 0 and t == 0)
                                last = (b_ == NBLK - 1 and k == 3
                                        and t == blk - 1)
                                for cc in range(2):
                                    for g in range(NG):
                                        m = 2 * g + cc
                                        if mode == "full":
                                            rhs = pk8[:, t * OS + m * CH:
                                                      t * OS + (m + 1) * CH]
                                        else:
                                            rhs = gbuf[:, m * CH:(m + 1) * CH]
                                        nc.tensor.matmul(
                                            acc[32 * g:32 * g + 8,
                                                cc * 512:cc * 512 + CH],
                                            lhsT, rhs,
                                            start=first, stop=last,
                                            tile_position=(0, 32 * g),
                                        )

                # ---------- epilogue ----------
                if mode == "full":
                    ot = opool.tile([128, 1024], dt.float32, tag="ot")
                    nc.vector.tensor_scalar(
                        out=ot[:], in0=acc[:], scalar1=g512[:], scalar2=sxg[:],
                        op0=AluOp.mult, op1=AluOp.subtract,
                    )
                    nc.vector.tensor_tensor(
                        out=ot[:], in0=ot[:], in1=biasc_t[:], op=AluOp.add,
                    )
                    for cc in range(2):
                        for g in range(NG):
                            m = 2 * g + cc
                            nc.sync.dma_start(
                                out[:, m * CH:(m + 1) * CH],
                                ot[32 * g:32 * g + 8,
                                   cc * 512:cc * 512 + CH],
                            )

            if mode != "full":
                nc.sync.dma_start(out[:, :], zt[:])

    nc.compile()
    return nc


def prepare_in_maps(inputs):
    """Host-side layout prep shared by kernel() and the perf harness."""
    x = np.asarray(inputs["x"], dtype=np.float32)
    packed_weight = np.asarray(inputs["packed_weight"], dtype=np.int32)
    weight_scale = np.asarray(inputs["weight_scale"], dtype=np.float32)
    bias = np.asarray(inputs["bias"], dtype=np.float32)

    # x -> stationary layout [p, (jt k b)]
    xs_np = np.ascontiguousarray(
        x.reshape(B, NJT, 128, 4).transpose(2, 1, 3, 0)
    ).reshape(128, 512)
    ws_np = weight_scale.reshape(1, 1)

    wb8 = packed_weight.astype(np.uint8)          # lossless: values <= 170
    in_maps = []
    for c in range(NCORES):
        sl = slice(c * OS, (c + 1) * OS)
        wbc = np.ascontiguousarray(wb8[sl, :].T)  # [JB, OS] uint8
        bc = bias[sl]
        biasc = np.zeros((128, 1024), dtype=np.float32)
        for g in range(NG):
            for cc in range(2):
                m = 2 * g + cc
                biasc[32 * g:32 * g + 8, cc * 512:cc * 512 + CH] = (
                    bc[m * CH:(m + 1) * CH][None, :]
                )
        in_maps.append({"wb": wbc, "xs": xs_np, "biasc": biasc, "ws": ws_np})
    return in_maps


def kernel(x, packed_weight, weight_scale, bias):
    global LAST_RESULTS
    repeat = int(os.environ.get("BITNET_REPEAT", "1"))
    mode = os.environ.get("BITNET_MODE", "full")
    blk = int(os.environ.get("BITNET_BLK", "4"))
    key = (repeat, mode, blk)
    if key not in _cache:
        _cache[key] = _build(repeat, mode, blk)
    nc = _cache[key]

    in_maps = prepare_in_maps(
        {"x": x, "packed_weight": packed_weight,
         "weight_scale": weight_scale, "bias": bias}
    )
    res = run_bass_kernel_spmd(nc, in_maps, list(range(NCORES)))
    LAST_RESULTS = res
    return np.concatenate(
        [np.asarray(res.results[c]["out"]) for c in range(NCORES)], axis=1
    ).reshape(B, O)
